# revision 35
# baseline (speedup 1.0000x reference)
"""
Bayesian categorical cross-entropy (Kendall & Gal) — Trainium2 Bass kernel.

Math: the reference perturbs logits with Gaussian noise whose std is
`true * sqrt(var)` — nonzero ONLY at the true class. So for sample b and
MC draw t, only the true-class logit moves:

    zt      = z_l + n_{t,b} * sqrt(var_l)
    CE_{t,b} = log(S_rest + exp(zt)) - zt,   S_rest = sum_c exp(z_c) - exp(z_l)

and the loss is mean_{t,b} CE. The full [T,B,C] tensors never need to be
materialized: per sample we need z_l, var_l (gathered at the true class),
S (row sum of exp over the logits), and the T standard-normal draws at the
true-class positions of the reference's fixed-seed noise tensor.

Sharding (data-parallel, per the hint): batch rows are split 256-per-core
across 8 NeuronCores; each core computes the mean CE of its shard on
device; the 8 partial means are averaged (the all-reduce-mean step).

Host-side prep is limited to index metadata and the fixed-seed PRNG:
 - labels = argmax(true) decodes the one-hot (index extraction);
 - flat gather offsets derived from labels;
 - the reference's noise at the true-class positions. The noise comes from
   jax.random.normal(key(42), (T,B,C)) — in this environment that is the
   'rbg' PRNG (XLA RngBitGenerator, backend-defined, not replicable in
   closed form), and its draws are a fixed-seed constant independent of
   the input values. We evaluate the same eager jax ops once and gather.
All arithmetic on the actual input values (pred_var) runs on-device:
S and exp via the ScalarEngine, z_l/var_l fetched by indirect DMA, the
T-sample CE and reductions on DVE/PE.
"""

import numpy as np

T = 100
C = 1000
B = 2048
N_CORES = 8
ROWS = B // N_CORES          # 256 batch rows per core
RT = ROWS // 128             # 2 row-tiles of 128 partitions per core

_cache = {}


def _noise_bt(labels: np.ndarray) -> np.ndarray:
    """[B, T] f32: reference noise gathered at the true-class index per row."""
    key = labels.tobytes()
    if key not in _cache:
        import jax
        import jax.numpy as jnp

        # Must mirror the reference's *eager* op sequence exactly: on this
        # backend the rbg RngBitGenerator output depends on the compiled
        # graph around it, so a jit-fused gather yields different draws.
        noise = jax.random.normal(jax.random.key(42), (T, B, C), jnp.float32)
        g = noise[:, jnp.arange(B), jnp.asarray(labels)]          # [T, B]
        _cache[key] = np.ascontiguousarray(np.asarray(g).T)       # [B, T]
        del noise, g
    return _cache[key]


def _build_nc():
    if "nc" in _cache:
        return _cache["nc"]
    import concourse.bass as bass
    import concourse.mybir as mybir
    import concourse.tile as tile
    import concourse.bacc as bacc_mod
    from concourse.bacc import Bacc

    f32 = mybir.dt.float32
    i32 = mybir.dt.int32
    AF = mybir.ActivationFunctionType
    OP = mybir.AluOpType

    # The act-table placement pass picks the FIRST act_info.json set that
    # contains each activation function, so Exp->set0 and Ln->set5 — every
    # Exp<->Ln switch then costs a ~1.3us LoadActFuncSet. All functions this
    # kernel uses (exp, ln, copy, identity) live together in the
    # natural_log_exp_and_others set; hide exp/ln from the other sets
    # (keeping set indices intact — walrus resolves the id against the same
    # act_info.json) so the whole kernel runs off one table load.
    if not getattr(bacc_mod, "_combined_act_tables_patch", False):
        _orig_tables = bacc_mod.get_activation_tables

        def _tables_combined(arch):
            t = _orig_tables(arch)
            AF_ = mybir.ActivationFunctionType
            return {
                name: (funcs if "exp" in name and "log" in name
                       else funcs - {AF_.Exp, AF_.Ln})
                for name, funcs in t.items()
            }

        bacc_mod.get_activation_tables = _tables_combined
        bacc_mod._combined_act_tables_patch = True

    nc = Bacc()
    # pa: [z(rows 0:128) | noise(rows 0:128) | noise(rows 128:256)],
    # pb: [z(rows 128:256)] — the two streamed chunks. pv: full pred_var,
    # touched only by the 4-element-per-partition gather. off: flat element
    # indices of (var_l, z_l) per row, staged to SBUF for the dynamic DMA.
    pa = nc.declare_dram_parameter("pa", [128, C + 2 * T], f32, isOutput=False)
    pb = nc.declare_dram_parameter("pb", [128, C], f32, isOutput=False)
    pv = nc.declare_dram_parameter("pv", [ROWS, 2 * C], f32, isOutput=False)
    off = nc.declare_dram_parameter("off", [128, 4], i32, isOutput=False)
    out = nc.declare_dram_parameter("out", [1, 1], f32, isOutput=True)

    pv_flat = pv[:].rearrange("r (c x) -> (r c) x", x=1)

    with tile.TileContext(nc) as tc:
        with (
            tc.tile_pool(name="pool", bufs=1) as pool,
            tc.tile_pool(name="psum", bufs=1, space=bass.MemorySpace.PSUM) as psum,
        ):
            # scaled ones: the PE dot then yields sum(ce)/(ROWS*T) directly
            ones = pool.tile([128, 1], f32)
            nc.vector.memset(ones[:], 1.0 / (ROWS * T))
            acc = psum.tile([1, 2], f32)

            # offsets must sit in SBUF for the HW dynamic-DMA; tiny transfer,
            # first on the SP ring so the gather chain starts earliest
            off_t = pool.tile([128, 4], i32)
            nc.sync.dma_start(off_t[:, :], off[:, :])
            # gzv[p] = (var_l[p], var_l[p+128], z_l[p], z_l[p+128]); HW
            # indirect DMA is row-granular (one index per partition), so one
            # gather per value. var first: the sqrt chain consumes it first.
            gzv = pool.tile([128, 4], f32)
            for k in range(4):
                nc.gpsimd.indirect_dma_start(
                    out=gzv[:, k:k + 1], out_offset=None,
                    in_=pv_flat,
                    in_offset=bass.IndirectOffsetOnAxis(
                        ap=off_t[:, k:k + 1], axis=0),
                )

            # stream order: block b halves first, block a (with the noise
            # columns) last — the tail after the last exp is then shortest
            pa_t = pool.tile([128, C + 2 * T], f32)
            pb_t = pool.tile([128, C], f32)
            half = C // 2
            nc.sync.dma_start(pb_t[:, 0:half], pb[:, 0:half])
            nc.sync.dma_start(pb_t[:, half:C], pb[:, half:C])
            nc.sync.dma_start(pa_t[:, 0:half], pa[:, 0:half])
            nc.sync.dma_start(pa_t[:, half:], pa[:, half:])

            # consolidate on DVE, one completion lane per copy: every later
            # consumer of zv then sees a single DVE semaphore
            zv = pool.tile([128, 4], f32)
            for k in range(4):
                nc.vector.tensor_copy(zv[:, k:k + 1], gzv[:, k:k + 1])

            e_sc = pool.tile([128, C], f32)
            s = pool.tile([128, 4], f32)
            lnv = pool.tile([128, 2], f32)
            sl = pool.tile([128, 2], f32)
            el = pool.tile([128, 2], f32)
            srest = pool.tile([128, 2], f32)
            junk = pool.tile([1, 2], f32)
            zt = pool.tile([128, 2 * T], f32)
            ez = pool.tile([128, 2 * T], f32)
            ll = pool.tile([128, 2 * T], f32)
            ced = pool.tile([128, 2 * T], f32)
            ce = pool.tile([128, 2], f32)
            fin = pool.tile([1, 1], f32)

            # ---- ACT stream; s layout: (s_b1, s_b2, s_a1, s_a2). z is O(5),
            # no max-shift needed. sqrt(v) = exp(0.5*ln(v)) keeps every ACT
            # function within the natural_log_exp_and_others table set: one
            # table load total.
            nc.scalar.activation(e_sc[:, 0:half], pb_t[:, 0:half], AF.Exp,
                                 accum_out=s[:, 0:1])
            nc.scalar.activation(e_sc[:, half:C], pb_t[:, half:C], AF.Exp,
                                 accum_out=s[:, 1:2])
            nc.scalar.activation(e_sc[:, 0:half], pa_t[:, 0:half], AF.Exp,
                                 accum_out=s[:, 2:3])
            nc.scalar.activation(lnv[:], zv[:, 0:2], AF.Ln)
            nc.scalar.activation(sl[:], lnv[:], AF.Exp, scale=0.5)
            nc.scalar.activation(e_sc[:, half:C], pa_t[:, half:C], AF.Exp,
                                 accum_out=s[:, 3:4])
            nc.scalar.activation(el[:], zv[:, 2:4], AF.Exp)

            # DVE witness for the pa chunk so zt below only adds the ACT wait
            nc.vector.tensor_copy(junk[0:1, 1:2], pa_t[0:1, C:C + 1])
            # zt = nz*sqrt(var_l) + z_l per row-block (scalars broadcast)
            nc.vector.tensor_scalar(
                out=zt[:, 0:T], in0=pa_t[:, C:C + T], scalar1=sl[:, 0:1],
                scalar2=zv[:, 2:3], op0=OP.mult, op1=OP.add)
            nc.vector.tensor_scalar(
                out=zt[:, T:2 * T], in0=pa_t[:, C + T:C + 2 * T],
                scalar1=sl[:, 1:2], scalar2=zv[:, 3:4],
                op0=OP.mult, op1=OP.add)
            nc.scalar.activation(ez[:], zt[:], AF.Exp)

            # S_rest per block: sum of the half-accumulators minus exp(z_l);
            # block b's S is ready long before block a's last exp, so its
            # whole CE tail is emitted first and overlaps exp_a2
            nc.vector.tensor_scalar(
                out=srest[:, 1:2], in0=s[:, 0:1], scalar1=s[:, 1:2],
                scalar2=el[:, 1:2], op0=OP.add, op1=OP.subtract)
            nc.scalar.activation(ll[:, T:2 * T], ez[:, T:2 * T], AF.Ln,
                                 bias=srest[:, 1:2])
            nc.vector.tensor_sub(ced[:, T:2 * T], ll[:, T:2 * T],
                                 zt[:, T:2 * T])
            nc.vector.tensor_reduce(ce[:, 1:2], ced[:, T:2 * T],
                                    axis=mybir.AxisListType.X, op=OP.add)
            nc.vector.tensor_scalar(
                out=srest[:, 0:1], in0=s[:, 2:3], scalar1=s[:, 3:4],
                scalar2=el[:, 0:1], op0=OP.add, op1=OP.subtract)
            nc.scalar.activation(ll[:, 0:T], ez[:, 0:T], AF.Ln,
                                 bias=srest[:, 0:1])
            nc.vector.tensor_sub(ced[:, 0:T], ll[:, 0:T], zt[:, 0:T])
            nc.vector.tensor_reduce(ce[:, 0:1], ced[:, 0:T],
                                    axis=mybir.AxisListType.X, op=OP.add)

            nc.tensor.matmul(acc[0:1, 0:2], ones[:], ce[:],
                             start=True, stop=True)
            nc.vector.tensor_reduce(fin[:], acc[0:1, 0:2],
                                    axis=mybir.AxisListType.X, op=OP.add)
            nc.sync.dma_start(out[0:1, 0:1], fin[:])

    nc.finalize()
    _cache["nc"] = nc
    return nc


def _pack_core(pv_j: np.ndarray, nz_j: np.ndarray, lab_j: np.ndarray) -> dict:
    """Build one core's input map from its [ROWS, 2C] pred_var shard, its
    [ROWS, T] noise shard and its [ROWS] labels (index metadata)."""
    fz = (np.arange(ROWS, dtype=np.int64) * (2 * C)
          + lab_j.astype(np.int64)).astype(np.int32)
    # per partition p: (var_l[p], var_l[p+128], z_l[p], z_l[p+128])
    off_j = np.stack([fz[0:128] + C, fz[128:256] + C,
                      fz[0:128], fz[128:256]], axis=1)
    pa_j = np.concatenate([pv_j[0:128, 0:C], nz_j[0:128], nz_j[128:256]],
                          axis=1)
    return {
        "pa": np.ascontiguousarray(pa_j),
        "pb": np.ascontiguousarray(pv_j[128:256, 0:C]),
        "pv": np.ascontiguousarray(pv_j),
        "off": np.ascontiguousarray(off_j),
    }


def kernel(true: np.ndarray, pred_var: np.ndarray) -> np.ndarray:
    from concourse.bass_utils import run_bass_kernel_spmd

    true = np.ascontiguousarray(true, dtype=np.float32)
    pred_var = np.ascontiguousarray(pred_var, dtype=np.float32)
    labels = np.argmax(true, axis=1).astype(np.int32)
    noise = _noise_bt(labels)

    nc = _build_nc()
    in_maps = []
    for j in range(N_CORES):
        r = slice(j * ROWS, (j + 1) * ROWS)
        in_maps.append(_pack_core(pred_var[r], noise[r], labels[r]))
    res = run_bass_kernel_spmd(nc, in_maps, list(range(N_CORES)))
    parts = np.array([res.results[j]["out"][0, 0] for j in range(N_CORES)],
                     dtype=np.float32)
    # all-reduce-mean across the 8 equal shards
    return np.asarray(parts.mean(), dtype=np.float32)


# revision 36
# speedup vs baseline: 1.0279x; 1.0279x over previous
"""
Bayesian categorical cross-entropy (Kendall & Gal) — Trainium2 Bass kernel.

Math: the reference perturbs logits with Gaussian noise whose std is
`true * sqrt(var)` — nonzero ONLY at the true class. So for sample b and
MC draw t, only the true-class logit moves:

    zt      = z_l + n_{t,b} * sqrt(var_l)
    CE_{t,b} = log(S_rest + exp(zt)) - zt,   S_rest = sum_c exp(z_c) - exp(z_l)

and the loss is mean_{t,b} CE. The full [T,B,C] tensors never need to be
materialized: per sample we need z_l, var_l (gathered at the true class),
S (row sum of exp over the logits), and the T standard-normal draws at the
true-class positions of the reference's fixed-seed noise tensor.

Sharding (data-parallel, per the hint): batch rows are split 256-per-core
across 8 NeuronCores; each core computes the mean CE of its shard on
device; the 8 partial means are averaged (the all-reduce-mean step).

Host-side prep is limited to index metadata and the fixed-seed PRNG:
 - labels = argmax(true) decodes the one-hot (index extraction);
 - flat gather offsets derived from labels;
 - the reference's noise at the true-class positions. The noise comes from
   jax.random.normal(key(42), (T,B,C)) — in this environment that is the
   'rbg' PRNG (XLA RngBitGenerator, backend-defined, not replicable in
   closed form), and its draws are a fixed-seed constant independent of
   the input values. We evaluate the same eager jax ops once and gather.
All arithmetic on the actual input values (pred_var) runs on-device:
S and exp via the ScalarEngine, z_l/var_l fetched by indirect DMA, the
T-sample CE and reductions on DVE/PE.
"""

import numpy as np

T = 100
C = 1000
B = 2048
N_CORES = 8
ROWS = B // N_CORES          # 256 batch rows per core
RT = ROWS // 128             # 2 row-tiles of 128 partitions per core

_cache = {}


def _noise_bt(labels: np.ndarray) -> np.ndarray:
    """[B, T] f32: reference noise gathered at the true-class index per row."""
    key = labels.tobytes()
    if key not in _cache:
        import jax
        import jax.numpy as jnp

        # Must mirror the reference's *eager* op sequence exactly: on this
        # backend the rbg RngBitGenerator output depends on the compiled
        # graph around it, so a jit-fused gather yields different draws.
        noise = jax.random.normal(jax.random.key(42), (T, B, C), jnp.float32)
        g = noise[:, jnp.arange(B), jnp.asarray(labels)]          # [T, B]
        _cache[key] = np.ascontiguousarray(np.asarray(g).T)       # [B, T]
        del noise, g
    return _cache[key]


def _build_nc():
    if "nc" in _cache:
        return _cache["nc"]
    import concourse.bass as bass
    import concourse.mybir as mybir
    import concourse.tile as tile
    import concourse.bacc as bacc_mod
    from concourse.bacc import Bacc

    f32 = mybir.dt.float32
    i32 = mybir.dt.int32
    AF = mybir.ActivationFunctionType
    OP = mybir.AluOpType

    # The act-table placement pass picks the FIRST act_info.json set that
    # contains each activation function, so Exp->set0 and Ln->set5 — every
    # Exp<->Ln switch then costs a ~1.3us LoadActFuncSet. All functions this
    # kernel uses (exp, ln, copy, identity) live together in the
    # natural_log_exp_and_others set; hide exp/ln from the other sets
    # (keeping set indices intact — walrus resolves the id against the same
    # act_info.json) so the whole kernel runs off one table load.
    if not getattr(bacc_mod, "_combined_act_tables_patch", False):
        _orig_tables = bacc_mod.get_activation_tables

        def _tables_combined(arch):
            t = _orig_tables(arch)
            AF_ = mybir.ActivationFunctionType
            return {
                name: (funcs if "exp" in name and "log" in name
                       else funcs - {AF_.Exp, AF_.Ln})
                for name, funcs in t.items()
            }

        bacc_mod.get_activation_tables = _tables_combined
        bacc_mod._combined_act_tables_patch = True

    nc = Bacc()
    # pa: [z(rows 0:128) | noise(rows 0:128) | noise(rows 128:256)],
    # pb: [z(rows 128:256)] — the two streamed chunks. pv: full pred_var,
    # touched only by the 4-element-per-partition gather. off: flat element
    # indices of (var_l, z_l) per row, staged to SBUF for the dynamic DMA.
    pa = nc.declare_dram_parameter("pa", [128, C + 2 * T], f32, isOutput=False)
    pb = nc.declare_dram_parameter("pb", [128, C], f32, isOutput=False)
    pv = nc.declare_dram_parameter("pv", [ROWS, 2 * C], f32, isOutput=False)
    off = nc.declare_dram_parameter("off", [128, 4], i32, isOutput=False)
    out = nc.declare_dram_parameter("out", [1, 1], f32, isOutput=True)

    pv_flat = pv[:].rearrange("r (c x) -> (r c) x", x=1)

    with tile.TileContext(nc) as tc:
        with (
            tc.tile_pool(name="pool", bufs=1) as pool,
            tc.tile_pool(name="psum", bufs=1, space=bass.MemorySpace.PSUM) as psum,
        ):
            # scaled ones: the PE dot then yields sum(ce)/(ROWS*T) directly
            ones = pool.tile([128, 1], f32)
            nc.vector.memset(ones[:], 1.0 / (ROWS * T))
            acc = psum.tile([1, 2], f32)

            # offsets must sit in SBUF for the HW dynamic-DMA; tiny transfer,
            # first on the SP ring so the gather chain starts earliest
            off_t = pool.tile([128, 4], i32)
            nc.sync.dma_start(off_t[:, :], off[:, :])
            # gzv[p] = (var_a, z_a, var_b, z_b); HW indirect DMA is
            # row-granular (one index per partition), so one gather per
            # value. Block a's pair first: its chain then starts a full
            # gather-receipt earlier than block b's.
            gzv = pool.tile([128, 4], f32)
            for k in range(4):
                nc.gpsimd.indirect_dma_start(
                    out=gzv[:, k:k + 1], out_offset=None,
                    in_=pv_flat,
                    in_offset=bass.IndirectOffsetOnAxis(
                        ap=off_t[:, k:k + 1], axis=0),
                )

            # stream order: block b halves first, block a (with the noise
            # columns) last — the tail after the last exp is then shortest
            pa_t = pool.tile([128, C + 2 * T], f32)
            pb_t = pool.tile([128, C], f32)
            half = C // 2
            nc.sync.dma_start(pb_t[:, 0:half], pb[:, 0:half])
            nc.sync.dma_start(pb_t[:, half:C], pb[:, half:C])
            nc.sync.dma_start(pa_t[:, 0:half], pa[:, 0:half])
            nc.sync.dma_start(pa_t[:, half:], pa[:, half:])

            # consolidate on DVE, one completion lane per copy: every later
            # consumer of zv then sees a single DVE semaphore
            zv = pool.tile([128, 4], f32)
            for k in range(4):
                nc.vector.tensor_copy(zv[:, k:k + 1], gzv[:, k:k + 1])

            e_sc = pool.tile([128, C], f32)
            s = pool.tile([128, 4], f32)
            lnv = pool.tile([128, 2], f32)
            sl = pool.tile([128, 2], f32)
            el = pool.tile([128, 2], f32)
            srest = pool.tile([128, 2], f32)
            junk = pool.tile([1, 2], f32)
            zt = pool.tile([128, 2 * T], f32)
            ez = pool.tile([128, 2 * T], f32)
            ll = pool.tile([128, 2 * T], f32)
            ced = pool.tile([128, 2 * T], f32)
            ce = pool.tile([128, 2], f32)
            fin = pool.tile([1, 1], f32)

            # ---- ACT stream; s layout: (s_b1, s_b2, s_a1, s_a2). z is O(5),
            # no max-shift needed. sqrt(v) = exp(0.5*ln(v)) keeps every ACT
            # function within the natural_log_exp_and_others table set: one
            # table load total.
            nc.scalar.activation(e_sc[:, 0:half], pb_t[:, 0:half], AF.Exp,
                                 accum_out=s[:, 0:1])
            nc.scalar.activation(e_sc[:, half:C], pb_t[:, half:C], AF.Exp,
                                 accum_out=s[:, 1:2])
            nc.scalar.activation(e_sc[:, 0:half], pa_t[:, 0:half], AF.Exp,
                                 accum_out=s[:, 2:3])
            nc.scalar.activation(lnv[:, 0:1], zv[:, 0:1], AF.Ln)
            nc.scalar.activation(sl[:, 0:1], lnv[:, 0:1], AF.Exp, scale=0.5)
            nc.scalar.activation(el[:, 0:1], zv[:, 1:2], AF.Exp)
            nc.scalar.activation(e_sc[:, half:C], pa_t[:, half:C], AF.Exp,
                                 accum_out=s[:, 3:4])
            nc.scalar.activation(lnv[:, 1:2], zv[:, 2:3], AF.Ln)
            nc.scalar.activation(sl[:, 1:2], lnv[:, 1:2], AF.Exp, scale=0.5)
            nc.scalar.activation(el[:, 1:2], zv[:, 3:4], AF.Exp)

            # DVE witness for the pa chunk so zt below only adds the ACT wait
            nc.vector.tensor_copy(junk[0:1, 1:2], pa_t[0:1, C:C + 1])
            # zt = nz*sqrt(var_l) + z_l per row-block (scalars broadcast)
            nc.vector.tensor_scalar(
                out=zt[:, 0:T], in0=pa_t[:, C:C + T], scalar1=sl[:, 0:1],
                scalar2=zv[:, 1:2], op0=OP.mult, op1=OP.add)
            nc.vector.tensor_scalar(
                out=zt[:, T:2 * T], in0=pa_t[:, C + T:C + 2 * T],
                scalar1=sl[:, 1:2], scalar2=zv[:, 3:4],
                op0=OP.mult, op1=OP.add)
            nc.scalar.activation(ez[:, 0:T], zt[:, 0:T], AF.Exp)
            nc.scalar.activation(ez[:, T:2 * T], zt[:, T:2 * T], AF.Exp)

            # S_rest per block; block a's whole chain is gated only by its
            # own two (early) gathers, so its tail runs first
            nc.vector.tensor_scalar(
                out=srest[:, 0:1], in0=s[:, 2:3], scalar1=s[:, 3:4],
                scalar2=el[:, 0:1], op0=OP.add, op1=OP.subtract)
            nc.scalar.activation(ll[:, 0:T], ez[:, 0:T], AF.Ln,
                                 bias=srest[:, 0:1])
            nc.vector.tensor_sub(ced[:, 0:T], ll[:, 0:T], zt[:, 0:T])
            nc.vector.tensor_reduce(ce[:, 0:1], ced[:, 0:T],
                                    axis=mybir.AxisListType.X, op=OP.add)
            nc.vector.tensor_scalar(
                out=srest[:, 1:2], in0=s[:, 0:1], scalar1=s[:, 1:2],
                scalar2=el[:, 1:2], op0=OP.add, op1=OP.subtract)
            nc.scalar.activation(ll[:, T:2 * T], ez[:, T:2 * T], AF.Ln,
                                 bias=srest[:, 1:2])
            nc.vector.tensor_sub(ced[:, T:2 * T], ll[:, T:2 * T],
                                 zt[:, T:2 * T])
            nc.vector.tensor_reduce(ce[:, 1:2], ced[:, T:2 * T],
                                    axis=mybir.AxisListType.X, op=OP.add)

            nc.tensor.matmul(acc[0:1, 0:2], ones[:], ce[:],
                             start=True, stop=True)
            nc.vector.tensor_reduce(fin[:], acc[0:1, 0:2],
                                    axis=mybir.AxisListType.X, op=OP.add)
            nc.sync.dma_start(out[0:1, 0:1], fin[:])

    nc.finalize()
    _cache["nc"] = nc
    return nc


def _pack_core(pv_j: np.ndarray, nz_j: np.ndarray, lab_j: np.ndarray) -> dict:
    """Build one core's input map from its [ROWS, 2C] pred_var shard, its
    [ROWS, T] noise shard and its [ROWS] labels (index metadata)."""
    fz = (np.arange(ROWS, dtype=np.int64) * (2 * C)
          + lab_j.astype(np.int64)).astype(np.int32)
    # per partition p: (var_a, z_a, var_b, z_b)
    off_j = np.stack([fz[0:128] + C, fz[0:128],
                      fz[128:256] + C, fz[128:256]], axis=1)
    pa_j = np.concatenate([pv_j[0:128, 0:C], nz_j[0:128], nz_j[128:256]],
                          axis=1)
    return {
        "pa": np.ascontiguousarray(pa_j),
        "pb": np.ascontiguousarray(pv_j[128:256, 0:C]),
        "pv": np.ascontiguousarray(pv_j),
        "off": np.ascontiguousarray(off_j),
    }


def kernel(true: np.ndarray, pred_var: np.ndarray) -> np.ndarray:
    from concourse.bass_utils import run_bass_kernel_spmd

    true = np.ascontiguousarray(true, dtype=np.float32)
    pred_var = np.ascontiguousarray(pred_var, dtype=np.float32)
    labels = np.argmax(true, axis=1).astype(np.int32)
    noise = _noise_bt(labels)

    nc = _build_nc()
    in_maps = []
    for j in range(N_CORES):
        r = slice(j * ROWS, (j + 1) * ROWS)
        in_maps.append(_pack_core(pred_var[r], noise[r], labels[r]))
    res = run_bass_kernel_spmd(nc, in_maps, list(range(N_CORES)))
    parts = np.array([res.results[j]["out"][0, 0] for j in range(N_CORES)],
                     dtype=np.float32)
    # all-reduce-mean across the 8 equal shards
    return np.asarray(parts.mean(), dtype=np.float32)
